# revision 52
# baseline (speedup 1.0000x reference)
"""Trainium2 Bass kernel for masked general attention (ragged sequences).

reference computation per batch b:
    q       = query[b] @ W_in.T                      [Lq, D]
    S       = q @ context[b].T                       [Lq, Lk]
    S_m     = where(qmask & kmask, S, -1e9)
    W       = softmax(S_m, axis=-1)
    mix     = W @ context[b]                         [Lq, D]
    out     = tanh(concat([mix, q]) @ W_out.T)       [Lq, D]
    returns (out, S_m)

Sharding / specialization strategy:
- Data-parallel over batch: 32 batches / 8 cores, SPMD (one program).
- W_in folded on the host: scores = query @ (W_in-projected context)
  (cw built per batch on device); the q-half of the output matmul uses
  Wfused = W_in.T @ W_out[:,D:].T so q is never materialized.
- Ragged-length specialization: batches assigned to 4 slots x 8 cores
  by simulated annealing on the baked cost; per slot only q-tiles /
  k-tiles under the slot max are computed.
- Fully-masked query rows need a uniform-softmax mix; where the slot
  has a spare masked k-row (cl_max < NKT*128) the context-mean rides
  that row of cn and the masked-row indicator is injected into the
  softmax weights, so no extra rank-1 matmul is spent.
- Whole matmul chain fp16; one packed fp16 query-block load feeds both
  the scores and the output matmul.  Scores go to DRAM as bf16 (host
  restores the exact -1e9 fill), output as fp16; never-computed score
  regions are padded host-side.
- Every bulk tensor is host-packed so each load is ONE long-row DMA
  (the per-DMA issue cost on the Sync queue is ~0.6us); output DMAs
  ride the otherwise-idle GpSimd queue.  Two-block scores lookahead
  bridges the serial softmax latency; the out-only blocks past each
  slot's query length are hoisted to the front as cold-start filler;
  wide warmup matmuls defeat the HAM cold clock.
"""

import sys

sys.path.insert(0, "/opt/trn_rl_repo")

import math
import random

import numpy as np

import concourse.bass as bass
import concourse.tile as tile
from concourse import bacc, mybir
from concourse import bass_utils
from concourse.masks import make_identity

F32 = mybir.dt.float32
FP16 = mybir.dt.float16
BF16 = mybir.dt.bfloat16

B, Lq, Lk, D = 32, 1024, 1024, 1024
N_CORES = 8
BPC = B // N_CORES          # batches (slots) per core
MQ = 256                    # queries per block
NBLK = Lq // MQ             # q-blocks per batch
NEG = -1e9
BIG = 3.0e38

_cache = {}


def _k_chunks(nkt):
    """Score-column chunk widths: each in [256, 512], covering nkt*128
    columns with minimal padding."""
    total = max(256, nkt * 128)
    if total <= 512:
        return [total]
    if total <= 768:
        return [total - 256, 256]
    return [512, total - 512]


def _slot_cost(nq_max, nk_max):
    nqb = (nq_max + 1) // 2
    S = sum(_k_chunks(nk_max))
    cw = 64 * S
    cw1 = 8192 * nk_max
    comp = nqb * (16 * S + 256 * nk_max + (nk_max + 8) * 2048)
    skip = (4 - nqb) * 18432
    return cw + cw1 + comp + skip


def _assign_slots(query_lengths, context_lengths):
    """Partition the 32 batches into 4 slots x 8 cores minimizing the
    baked per-slot cost. Returns perm[slot][core] -> batch index and the
    per-slot (NQB, NKN, NKT, SPARE)."""
    nqt = -(-query_lengths.astype(np.int64) // 128)
    nkt = -(-context_lengths.astype(np.int64) // 128)

    def total(ss):
        return sum(_slot_cost(max(nqt[i] for i in s), max(nkt[i] for i in s))
                   for s in ss)

    gbest = None
    gslots = None
    for seed in range(3):
        order = np.argsort(nqt * nkt)
        slots = [list(order[j * N_CORES:(j + 1) * N_CORES])
                 for j in range(BPC)]
        cur = best = total(slots)
        bslots = [list(s) for s in slots]
        rng = random.Random(seed)
        iters = 120000
        for it in range(iters):
            T = 3000.0 * (1.0 - it / iters) + 1.0
            a, b = rng.randrange(BPC), rng.randrange(BPC)
            if a == b:
                continue
            i, j = rng.randrange(N_CORES), rng.randrange(N_CORES)
            slots[a][i], slots[b][j] = slots[b][j], slots[a][i]
            c = total(slots)
            if c <= cur or rng.random() < math.exp((cur - c) / T):
                cur = c
                if c < best:
                    best = c
                    bslots = [list(s) for s in slots]
            else:
                slots[a][i], slots[b][j] = slots[b][j], slots[a][i]
        if gbest is None or best < gbest:
            gbest = best
            gslots = bslots

    keyed = []
    for s in gslots:
        nq = max(nqt[i] for i in s)
        nk = max(nkt[i] for i in s)
        maxcl = max(int(context_lengths[i]) for i in s)
        spare = int(nk < 8 and maxcl < nk * 128)
        keyed.append(((int((nk + 3) // 4), int(nk), int((nq + 1) // 2)),
                      spare, s))
    keyed.sort(key=lambda kv: kv[0])
    slots = [s for _, _, s in keyed]
    params = tuple((k[2], k[0], k[1], sp) for k, sp, _ in keyed)
    return slots, params


def _build_program(params):
    """params: tuple of (NQB, NKN, NKT, SPARE) per slot."""
    nc = bacc.Bacc("TRN2", target_bir_lowering=False, debug=False,
                   num_devices=N_CORES)

    qblk_d = nc.dram_tensor("qblk", [BPC, NBLK, 128, 8 * MQ], FP16,
                            kind="ExternalInput").ap()
    cT_d = nc.dram_tensor("cT", [BPC, 128, 8 * Lk], FP16, kind="ExternalInput").ap()
    win_d = nc.dram_tensor("win", [128, 8 * D], FP16, kind="ExternalInput").ap()
    wo1_d = nc.dram_tensor("wo1", [128, 8 * D], FP16, kind="ExternalInput").ap()
    wf_d = nc.dram_tensor("wf", [128, 8 * D], FP16, kind="ExternalInput").ap()
    kmin_d = nc.dram_tensor("kmin", [BPC, 128, Lk], F32, kind="ExternalInput").ap()
    qq_d = nc.dram_tensor("qq", [BPC, 128, 24], F32, kind="ExternalInput").ap()
    m01_d = nc.dram_tensor("m01", [BPC, Lq], FP16, kind="ExternalInput").ap()
    cball_d = nc.dram_tensor("cball", [1, BPC * D], FP16, kind="ExternalInput").ap()

    out_d = nc.dram_tensor("out", [BPC, Lq, D], FP16, kind="ExternalOutput").ap()
    sc_d = nc.dram_tensor("sc", [BPC, Lq, Lk], BF16, kind="ExternalOutput").ap()

    with tile.TileContext(nc) as tc:
        with (
            tc.tile_pool(name="static", bufs=1) as st,
            tc.tile_pool(name="ctx1", bufs=1) as ctx1_pool,
            tc.tile_pool(name="ctx2", bufs=2) as ctx2_pool,
            tc.tile_pool(name="q16", bufs=3) as q16_pool,
            tc.tile_pool(name="q16s", bufs=2) as q16s_pool,
            tc.tile_pool(name="ew", bufs=3) as ew_pool,
            tc.tile_pool(name="wm", bufs=2) as wm_pool,
            tc.tile_pool(name="sm", bufs=3) as sm_pool,
            tc.tile_pool(name="sc16", bufs=4) as sc16_pool,
            tc.tile_pool(name="ot", bufs=3) as ot_pool,
            tc.tile_pool(name="stats", bufs=6) as stats_pool,
            tc.tile_pool(name="psS", bufs=3, space="PSUM") as psS,
            tc.tile_pool(name="psO", bufs=3, space="PSUM") as psO,
            tc.tile_pool(name="psT", bufs=2, space="PSUM") as psT,
        ):

            # ---- PE warmup: wide matmuls (LDWEIGHTS fully overlapped) so
            # the HAM activity window reads continuously-busy and releases
            # the full 2.4GHz clock before the first real matmul.
            warm_sb = st.tile([128, 512], FP16, tag="warm")
            nc.vector.memset(warm_sb[:], 0.25)
            for _ in range(24):
                pw = psT.tile([128, 512], F32, tag="psT")
                nc.tensor.matmul(pw[:], warm_sb[:, :128], warm_sb[:],
                                 start=True, stop=True)

            def qry16_dma(b, blk_i, pool=None, tag="q16"):
                pool = pool or q16_pool
                t = pool.tile([128, 8 * MQ], FP16, tag=tag)
                nc.sync.dma_start(t[:], qblk_d[b, blk_i])
                return t

            def load_cT(b):
                """Scores-side context, host-packed to one long-row DMA."""
                NQB, NKN, NKT, SPARE = params[b]
                S = sum(_k_chunks(NKT))
                cT_sb = ctx1_pool.tile([128, 8 * Lk], FP16, tag="cT")
                nc.sync.dma_start(cT_sb[:, :8 * S], cT_d[b, :, :8 * S])
                return dict(cT=cT_sb, S=S)

            def load_masks(b, ctx):
                NQB, NKN, NKT, SPARE = params[b]
                S = sum(_k_chunks(NKT))
                kmin_sb = ctx2_pool.tile([128, Lk], F32, tag="kmin")
                nc.sync.dma_start(kmin_sb[:, :S], kmin_d[b, :, :S])
                qq_sb = ctx2_pool.tile([128, 24], F32, tag="qq")
                nc.sync.dma_start(qq_sb[:], qq_d[b])
                ctx.update(kmin=kmin_sb, qq=qq_sb)

            def load_ctx_late(b, ctx):
                """m01 rank-1 fallback row: its tile slot is released only
                by the previous fallback batch's last out, so it is emitted
                after that out to keep the in-order DMA queue satisfiable."""
                NQB, NKN, NKT, SPARE = params[b]
                if not SPARE and NKT < 8:
                    m01_sb = ctx1_pool.tile([1, Lq], FP16, tag="m01")
                    nc.sync.dma_start(m01_sb[:], m01_d[b:b + 1, :])
                    ctx["m01"] = m01_sb

            def cw_build(b, ctx):
                """cw[d, k] = sum_e W_in[e, d] * contextT[e, k] (fp16)."""
                NQB, NKN, NKT, SPARE = params[b]
                S = ctx["S"]
                cw_sb = ctx2_pool.tile([128, 8 * Lk], FP16, tag="cw")
                for dt in range(8):
                    off = 0
                    for w in _k_chunks(NKT):
                        ps = psS.tile([128, 512], F32, tag="psS")
                        for et in range(8):
                            nc.tensor.matmul(
                                ps[:, :w],
                                win_sb[:, et * D + dt * 128:et * D + (dt + 1) * 128],
                                ctx["cT"][:, et * S + off:et * S + off + w],
                                start=(et == 0), stop=(et == 7))
                        nc.vector.tensor_copy(
                            cw_sb[:, dt * Lk + off:dt * Lk + off + w],
                            ps[:, :w])
                        off += w
                ctx["cw"] = cw_sb

            def cw1_build(b, ctx):
                """cw1[k, c] = sum_d context[k, d] * Wo1[d, c]: the mix
                matmul pre-folded through W_out's mix half, so the output
                stage consumes softmax weights directly."""
                NQB, NKN, NKT, SPARE = params[b]
                S = ctx["S"]
                cw1_sb = ctx2_pool.tile([128, 8 * D], FP16, tag="cw1")
                for kt in range(NKT):
                    for n2 in range(2):
                        ps = psS.tile([128, 512], F32, tag="psS")
                        for et in range(8):
                            nc.tensor.matmul(
                                ps[:],
                                ctx["cT"][:, et * S + kt * 128:et * S + (kt + 1) * 128],
                                wo1_sb[:, et * D + n2 * 512:et * D + (n2 + 1) * 512],
                                start=(et == 0), stop=(et == 7))
                        nc.vector.tensor_copy(
                            cw1_sb[:, kt * D + n2 * 512:kt * D + (n2 + 1) * 512],
                            ps[:])
                ctx["cw1"] = cw1_sb

            # [1, 128] ones row for the constant-output rank-1 matmul
            ones_sb = st.tile([1, 128], FP16, tag="ones")
            nc.vector.memset(ones_sb[:], 1.0)

            def scores_softmax(b, blk_i, qry_sb, ctx):
                """Masked scores -> DRAM (bf16); softmax weights -> ew."""
                NQB, NKN, NKT, SPARE = params[b]
                chunks = _k_chunks(NKT)
                S = sum(chunks)
                NCH = len(chunks)
                q0 = blk_i * MQ
                ew_sb = ew_pool.tile([128, 2 * Lk], FP16, tag="ew")
                for h in range(2):
                    jt = blk_i * 2 + h
                    rows = slice(q0 + h * 128, q0 + (h + 1) * 128)
                    stt = stats_pool.tile([128, 8], F32, tag="stats")
                    sm_n = []
                    off = 0
                    for n, w in enumerate(chunks):
                        ps = psS.tile([128, 512], F32, tag="psS")
                        for dt in range(8):
                            nc.tensor.matmul(
                                ps[:, :w],
                                qry_sb[:, dt * MQ + h * 128:dt * MQ + (h + 1) * 128],
                                ctx["cw"][:, dt * Lk + off:dt * Lk + off + w],
                                start=(dt == 0), stop=(dt == 7))
                        sm = sm_pool.tile([128, 512], F32, tag="sm")
                        sm_n.append((sm, off, w))
                        nc.vector.tensor_tensor(
                            sm[:, :w], ps[:, :w], ctx["kmin"][:, off:off + w],
                            op=mybir.AluOpType.min)
                        if NKT == 8:
                            # NKT<8 slots skip the q-mask min: masked-row
                            # weights are zeroed via q01 and their sc rows
                            # are restored host-side
                            nc.vector.tensor_scalar_min(
                                sm[:, :w], sm[:, :w], ctx["qq"][:, jt:jt + 1])
                        nc.vector.reduce_max(
                            stt[:, n:n + 1], sm[:, :w],
                            axis=mybir.AxisListType.X, negate=True)
                        off += w
                    if NCH == 1:
                        negm = stt[:, 0:1]
                    else:
                        nc.vector.tensor_tensor(
                            stt[:, 2:3], stt[:, 0:1], stt[:, 1:2],
                            op=mybir.AluOpType.min)
                        negm = stt[:, 2:3]
                    for n, (sm, off, w) in enumerate(sm_n):
                        nc.scalar.activation(
                            ew_sb[:, h * Lk + off:h * Lk + off + w],
                            sm[:, :w],
                            mybir.ActivationFunctionType.Exp,
                            bias=negm, scale=1.0,
                            accum_out=stt[:, 3 + n:4 + n])
                    if NCH == 1:
                        ssum = stt[:, 3:4]
                    else:
                        nc.vector.tensor_tensor(
                            stt[:, 5:6], stt[:, 3:4], stt[:, 4:5],
                            op=mybir.AluOpType.add)
                        ssum = stt[:, 5:6]
                    nc.vector.reciprocal(stt[:, 6:7], ssum)
                    if NKT == 8:
                        scale = stt[:, 6:7]
                    else:
                        # zero the weights of fully-masked query rows; their
                        # uniform mix re-enters via the spare cn row (or the
                        # rank-1 fallback matmul in the mix stage)
                        nc.vector.tensor_tensor(
                            stt[:, 7:8], stt[:, 6:7], ctx["qq"][:, 8 + jt:9 + jt],
                            op=mybir.AluOpType.mult)
                        scale = stt[:, 7:8]
                    nc.vector.tensor_scalar_mul(
                        ew_sb[:, h * Lk:h * Lk + S],
                        ew_sb[:, h * Lk:h * Lk + S],
                        scale)
                    if SPARE:
                        # masked-row indicator into the spare k-row: its cn
                        # row holds the context mean
                        r = NKT * 128 - 1
                        nc.vector.tensor_copy(
                            ew_sb[:, h * Lk + r:h * Lk + r + 1],
                            ctx["qq"][:, 16 + jt:17 + jt])
                    # bf16 scores for DRAM: cast on ACT after the exps so it
                    # never gates the softmax chain; single long-row DMA on
                    # the GpSimd queue so writes can't block input loads.
                    sc16 = sc16_pool.tile([128, 1024], BF16, tag="sc16")
                    for n, (sm, off, w) in enumerate(sm_n):
                        nc.scalar.activation(
                            sc16[:, off:off + w], sm[:, :w],
                            mybir.ActivationFunctionType.Copy)
                    nc.gpsimd.dma_start(sc_d[b, rows, :S], sc16[:, :S])
                return ew_sb

            def transposes(b, ew_sb, ident):
                NQB, NKN, NKT, SPARE = params[b]
                wt_sb = wm_pool.tile([128, 8 * MQ], FP16, tag="wm")
                for kt in range(NKT):
                    pt = psT.tile([128, MQ], FP16, tag="psT")
                    for h in range(2):
                        nc.tensor.transpose(
                            pt[:, h * 128:(h + 1) * 128],
                            ew_sb[:, h * Lk + kt * 128:h * Lk + (kt + 1) * 128],
                            ident[:])
                    nc.vector.tensor_copy(wt_sb[:, kt * MQ:(kt + 1) * MQ], pt[:])
                return wt_sb

            def out_stage(b, blk_i, q16_sb, wt_sb, ctx):
                NQB, NKN, NKT, SPARE = params[b]
                rank1 = (NKT < 8 and not SPARE)
                q0 = blk_i * MQ
                for h in range(2):
                    rows = slice(q0 + h * 128, q0 + (h + 1) * 128)
                    ot = ot_pool.tile([128, 1024], FP16, tag="ot")
                    for n in range(2):
                        po = psO.tile([128, 512], F32, tag="psO")
                        for kt in range(NKT):
                            nc.tensor.matmul(
                                po[:],
                                wt_sb[:, kt * MQ + h * 128:kt * MQ + (h + 1) * 128],
                                ctx["cw1"][:, kt * D + n * 512:kt * D + (n + 1) * 512],
                                start=(kt == 0), stop=False)
                        for dt in range(8):
                            nc.tensor.matmul(
                                po[:],
                                q16_sb[:, dt * MQ + h * 128:dt * MQ + (h + 1) * 128],
                                wf_sb[:, dt * D + n * 512:dt * D + (n + 1) * 512],
                                start=False, stop=(dt == 7 and not rank1))
                        if rank1:
                            nc.tensor.matmul(
                                po[:], ctx["m01"][0:1, rows],
                                cb_sbs[b][0:1, n * 512:(n + 1) * 512],
                                start=False, stop=True)
                        nc.scalar.activation(
                            ot[:, n * 512:(n + 1) * 512], po[:],
                            mybir.ActivationFunctionType.Tanh)
                    nc.gpsimd.dma_start(out_d[b, rows, :], ot[:])

            def skipped_block(b, blk_i):
                """q-block past every query length in the slot: scores are
                all -1e9 (host pads); out = tanh(query@Wfused + mean_ctx@Wo1).
                Emitted up front as cold-start PE filler."""
                q0 = blk_i * MQ
                q16_sb = qry16_dma(b, blk_i, pool=q16s_pool, tag="q16s")
                for h in range(2):
                    rows = slice(q0 + h * 128, q0 + (h + 1) * 128)
                    ot = ot_pool.tile([128, 1024], FP16, tag="ot")
                    for n in range(2):
                        po = psO.tile([128, 512], F32, tag="psO")
                        for dt in range(8):
                            nc.tensor.matmul(
                                po[:],
                                q16_sb[:, dt * MQ + h * 128:dt * MQ + (h + 1) * 128],
                                wf_sb[:, dt * D + n * 512:dt * D + (n + 1) * 512],
                                start=(dt == 0), stop=False)
                        nc.tensor.matmul(
                            po[:], ones_sb[0:1, :],
                            cb_sbs[b][0:1, n * 512:(n + 1) * 512],
                            start=False, stop=True)
                        nc.scalar.activation(
                            ot[:, n * 512:(n + 1) * 512], po[:],
                            mybir.ActivationFunctionType.Tanh)
                    nc.gpsimd.dma_start(out_d[b, rows, :], ot[:])

            # ---- prologue (load order tracks first-use order on the PE:
            # cw needs cT+win, cw1 needs wo1, scores need qry0)
            ctx0 = load_cT(0)
            win_sb = st.tile([128, 8 * D], FP16, tag="win")
            nc.sync.dma_start(win_sb[:], win_d[:, :])
            wo1_sb = st.tile([128, 8 * D], FP16, tag="wo1")
            nc.sync.dma_start(wo1_sb[:], wo1_d[:, :])
            qry0 = qry16_dma(0, 0)
            load_masks(0, ctx0)
            load_ctx_late(0, ctx0)
            wf_sb = st.tile([128, 8 * D], FP16, tag="wf")
            nc.sync.dma_start(wf_sb[:], wf_d[:, :])
            cball_sb = st.tile([1, BPC * D], FP16, tag="cball")
            nc.sync.dma_start(cball_sb[:], cball_d[0:1, :])
            ident = st.tile([128, 128], FP16, tag="ident")
            make_identity(nc, ident[:])
            cb_sbs = [cball_sb[0:1, j * D:(j + 1) * D] for j in range(BPC)]

            cw_build(0, ctx0)
            cw1_build(0, ctx0)

            # flattened computed-block sequence with TWO-block lookahead:
            # scores(i+1) and scores(i+2) are on the PE queue before
            # transposes(i), so the PE has ~2 blocks of work to bridge the
            # serial softmax chain on DVE/ACT.
            seq = [(b, i) for b in range(BPC) for i in range(params[b][0])]
            ctxs = {0: ctx0}

            def emit_scores(pos):
                eb, ei = seq[pos]
                ctx = ctxs[eb]
                if "cw" not in ctx:
                    if "kmin" not in ctx:
                        load_masks(eb, ctx)
                    cw_build(eb, ctx)
                    # cw1 right after cw: cT frees immediately, so the next
                    # batch's cT prefetch never head-blocks the Sync queue
                    cw1_build(eb, ctx)
                q = qry16_dma(eb, ei)
                sew = scores_softmax(eb, ei, q, ctx)
                return (eb, ei, q, sew)

            pend = [(0, 0, qry0, scores_softmax(0, 0, qry0, ctx0))]
            if len(seq) > 1:
                if seq[1][0] not in ctxs:
                    ctxs[seq[1][0]] = load_cT(seq[1][0])
                pend.append(emit_scores(1))
            # out-only blocks: PE filler while the first softmax chains run
            for sb_ in range(BPC):
                for si in range(params[sb_][0], NBLK):
                    skipped_block(sb_, si)
            for idx in range(len(seq)):
                b, i = seq[idx]
                # prefetch cT (and masks) three blocks before the crossing
                if idx + 3 < len(seq) and seq[idx + 3][0] not in ctxs:
                    fb = seq[idx + 3][0]
                    ctxs[fb] = load_cT(fb)
                    load_masks(fb, ctxs[fb])
                if idx + 2 < len(seq):
                    if seq[idx + 2][0] not in ctxs:
                        ctxs[seq[idx + 2][0]] = load_cT(seq[idx + 2][0])
                    pend.append(emit_scores(idx + 2))
                _, _, q16_sb, ew_sb = pend.pop(0)
                ctx = ctxs[b]
                wt_sb = transposes(b, ew_sb, ident)
                out_stage(b, i, q16_sb, wt_sb, ctx)
                nxt = seq[idx + 1] if idx + 1 < len(seq) else None
                if nxt is not None and nxt[0] != b:
                    load_ctx_late(nxt[0], ctxs[nxt[0]])

    nc.compile()
    return nc


def kernel(query, context, query_lengths, context_lengths, W_in, W_out):
    slots, params = _assign_slots(np.asarray(query_lengths),
                                  np.asarray(context_lengths))
    if _cache.get("params") != params:
        _cache["nc"] = _build_program(params)
        _cache["params"] = params
    nc = _cache["nc"]

    # batch order: core c processes batches [slots[0][c], slots[1][c], ...]
    perm = np.array(slots)                       # [BPC, N_CORES]
    flat = perm.T.reshape(-1)                    # core-major batch order

    query = np.asarray(query, dtype=np.float32)
    context = np.asarray(context, dtype=np.float32)
    ql = np.asarray(query_lengths).astype(np.int64)
    cl = np.asarray(context_lengths).astype(np.int64)

    # packed per-block query tiles: qblk[b, blk, p, dt*MQ+j]
    #   = query[b, blk*MQ+j, dt*128+p]
    qblk = np.ascontiguousarray(
        query.reshape(B, NBLK, MQ, 8, 128).transpose(0, 1, 4, 3, 2)
        .reshape(B, NBLK, 128, 8 * MQ)).astype(np.float16)
    win_f = np.asarray(W_in, dtype=np.float32)
    # packed weights: [p, et*D + dt*128 + c] = W[et*128+p, dt*128+c]
    win = np.ascontiguousarray(
        win_f.reshape(8, 128, 8, 128).transpose(1, 0, 2, 3)
        .reshape(128, 8 * D)).astype(np.float16)
    woT = np.ascontiguousarray(W_out.T, dtype=np.float32)
    wo1 = np.ascontiguousarray(
        woT[:D].reshape(8, 128, D).transpose(1, 0, 2)
        .reshape(128, 8 * D)).astype(np.float16)
    wf_f = (W_in.astype(np.float64).T @ woT[D:].astype(np.float64))
    wf = np.ascontiguousarray(
        wf_f.reshape(8, 128, D).transpose(1, 0, 2)
        .reshape(128, 8 * D)).astype(np.float16)
    mean_c64 = context.astype(np.float64).mean(axis=1)         # [B, D]
    cb = (mean_c64 @ woT[:D].astype(np.float64)).astype(np.float16)
    mean_c = mean_c64.astype(np.float16)

    # context with the mean injected into the slot's spare masked k-row
    # (rides into BOTH cw -- masked in scores -- and cw1, where it
    # provides the uniform-mix correction row)
    slot_of = np.empty(B, dtype=np.int64)
    for j, s in enumerate(slots):
        for i in s:
            slot_of[i] = j
    ctx_inj = context.copy()
    for bidx in range(B):
        NQB, NKN, NKT, SPARE = params[slot_of[bidx]]
        if SPARE:
            ctx_inj[bidx, NKT * 128 - 1, :] = mean_c64[bidx]
    # packed transposed context for the scores stage, per-slot S stride:
    # [b, p, et*S + k] = context[b, k, et*128 + p]
    cTfull = np.ascontiguousarray(
        ctx_inj.transpose(0, 2, 1).reshape(B, 8, 128, Lk)
        .transpose(0, 2, 1, 3))                  # [B, p, et, k]
    cT = np.zeros((B, 128, 8 * Lk), dtype=np.float16)
    for bidx in range(B):
        NQB, NKN, NKT, SPARE = params[slot_of[bidx]]
        S = sum(_k_chunks(NKT))
        cT[bidx, :, :8 * S] = (
            cTfull[bidx, :, :, :S].reshape(128, 8 * S).astype(np.float16))

    k_idx = np.arange(Lk)
    q_idx = np.arange(Lq)
    kvalid = k_idx[None, :] < cl[:, None]
    qvalid = q_idx[None, :] < ql[:, None]
    kmin = np.where(kvalid, np.float32(BIG), np.float32(NEG)).astype(np.float32)
    qmin = np.where(qvalid, np.float32(BIG), np.float32(NEG)).astype(np.float32)
    q01 = qvalid.astype(np.float32)
    m01f = (~qvalid).astype(np.float32)
    m01 = (~qvalid).astype(np.float16)
    kmin_rep = np.ascontiguousarray(
        np.broadcast_to(kmin[:, None, :], (B, 128, Lk)))
    # qq: per-jt columns [p, 0:8]=qmin, [8:16]=q01, [16:24]=m01
    qq = np.concatenate([
        qmin.reshape(B, 8, 128).transpose(0, 2, 1),
        q01.reshape(B, 8, 128).transpose(0, 2, 1),
        m01f.reshape(B, 8, 128).transpose(0, 2, 1)], axis=2)
    qq = np.ascontiguousarray(qq).astype(np.float32)

    in_maps = []
    for c in range(N_CORES):
        s = flat[c * BPC:(c + 1) * BPC]
        in_maps.append({
            "qblk": np.ascontiguousarray(qblk[s]),
            "cT": np.ascontiguousarray(cT[s]),
            "win": win, "wo1": wo1, "wf": wf,
            "kmin": np.ascontiguousarray(kmin_rep[s]),
            "qq": np.ascontiguousarray(qq[s]),
            "m01": np.ascontiguousarray(m01[s]),
            "cball": np.ascontiguousarray(cb[s].reshape(1, BPC * D)),
        })

    res = bass_utils.run_bass_kernel_spmd(nc, in_maps, core_ids=list(range(N_CORES)))
    _cache["last_result"] = res

    # bf16-rounded -1e9 (what the device wrote in masked slots): f32 ->
    # bf16 is round-to-nearest-even on the top 16 bits
    u = int(np.float32(NEG).view(np.uint32))
    lower, upper = u & 0xFFFF, u >> 16
    if lower > 0x8000 or (lower == 0x8000 and (upper & 1)):
        upper += 1
    neg_bf16_as_f32 = np.uint32(upper << 16).view(np.float32)

    out = np.empty((B, Lq, D), dtype=np.float32)
    scores = np.full((B, Lq, Lk), np.float32(NEG), dtype=np.float32)
    for c in range(N_CORES):
        s = flat[c * BPC:(c + 1) * BPC]
        out[s] = res.results[c]["out"].astype(np.float32)
        sc_dev = res.results[c]["sc"]
        for j, bidx in enumerate(s):
            NQB, NKN, NKT, SPARE = params[j]
            S = sum(_k_chunks(NKT))
            rows = MQ * NQB
            blk = sc_dev[j, :rows, :S].astype(np.float32)
            blk[blk == neg_bf16_as_f32] = np.float32(NEG)
            if NKT < 8 and ql[bidx] < rows:
                # device skips the q-mask min for these slots
                blk[ql[bidx]:, :] = np.float32(NEG)
            scores[bidx, :rows, :S] = blk
    return out, scores


_program_cache = _cache  # test.py compatibility
